# revision 42
# baseline (speedup 1.0000x reference)
"""Trainium2 Bass kernel for nn_Attention_40690520163106.

Multi-head causal attention with RoPE + LoRA on 8 NeuronCores.
Sharding: tensor-parallel over the 16 heads (2 heads/core), data-replicated
over batch; AllToAll reshard before the output projection so each core
computes a disjoint token slice of the final output (no reduction needed).

v2: host-side layout transforms (transposed x/weights, prebuilt rope tables),
SWDGE cast-loads, fast softmax normalize, and projection/output-projection
work interleaved into the attention loop so the PE array never idles.

Self-contained: hardcodes all shapes; reads nothing from /root/problem.
"""

import sys
import numpy as np

for _p in ("/opt/trn_rl_repo", "/root/.axon_site/_ro/trn_rl_repo"):
    if _p not in sys.path:
        sys.path.insert(0, _p)

import concourse.bass as bass
import concourse.mybir as mybir
import concourse.tile as tile
from concourse import bacc
from concourse.bass_utils import run_bass_kernel_spmd
from concourse.masks import make_identity

F32 = mybir.dt.float32
BF16 = mybir.dt.bfloat16
EXP = mybir.ActivationFunctionType.Exp
ADD = mybir.AluOpType.add
MULT = mybir.AluOpType.mult

B, S, D, H, HD, R = 2, 4096, 1024, 16, 64, 16
NCORES = 8
TOK = B * S
QT_TILE = 512               # q free-dim tile (one psum bank of fp32)
NQ = S // QT_TILE           # 8 q-tiles per batch
NKT = S // 128              # 32 k-tiles per batch
QUARTER = 2048              # tokens per projection quarter
NCHUNK = D // 128           # 8 contraction chunks

TRACE = False
LAST_EXEC_NS = None

_CACHE = {}


def _emit(nc, tc, io):
    import os as _os
    _ph = int(_os.environ.get("KPH", "99"))
    _dbg_all = _os.environ.get("KDBG", "") == "all"

    persist_ctx = tc.tile_pool(name="persist", bufs=1)
    persist_pool = persist_ctx.__enter__()
    sb1 = lambda shape, dt, name: persist_pool.tile(shape, dt, name=name, tag=name)

    # ---------------- persistent SBUF tensors ----------------
    ident_bf = sb1([128, 128], BF16, "ident_bf")
    make_identity(nc, ident_bf[:])

    wT = {nm: sb1([128, NCHUNK, 128], BF16, f"w{nm}T") for nm in ("q", "k", "v")}
    woT = sb1([128, NCHUNK, 1024], BF16, "woT")
    wq_b_sb = sb1([128, 1], F32, "wq_b_sb")
    wo_b_sb = sb1([1, 1024], BF16, "wo_b_sb")
    ones_row = sb1([1, 128], BF16, "ones_row")
    nc.vector.memset(ones_row[:], 1.0)
    tri8T = sb1([128, 128], F32, "tri8T")
    cosT4 = sb1([128, S], BF16, "cosT4")
    sinT4 = sb1([128, S], BF16, "sinT4")
    dbgA = sb1([128, 512], F32, "dbgA")
    dbgB = sb1([128, 512], F32, "dbgB")
    dbgC = sb1([128, 512], F32, "dbgC")
    l1 = {nm: sb1([16, 1024], BF16, f"l1{nm}") for nm in ("q", "k", "v", "o")}
    l2T = {nm: sb1([16, 128], BF16, f"l2T{nm}") for nm in ("q", "k", "v")}
    l2To = sb1([16, 1024], BF16, "l2To")

    with tc.tile_pool(name="ps_sm", bufs=1, space="PSUM") as ps_sm, \
         tc.tile_pool(name="ps_blip", bufs=1, space="PSUM") as ps_blip, \
         tc.tile_pool(name="ps_big", bufs=2, space="PSUM") as ps_big, \
         tc.tile_pool(name="ps_ot", bufs=2, space="PSUM") as ps_ot, \
         tc.tile_pool(name="xt", bufs=16) as xt_pool, \
         tc.tile_pool(name="qkv", bufs=2) as qkv_pool, \
         tc.tile_pool(name="rope", bufs=2) as rope_pool, \
         tc.tile_pool(name="pt", bufs=6) as pt_pool, \
         tc.tile_pool(name="norm", bufs=2) as norm_pool, \
         tc.tile_pool(name="normsm", bufs=3) as normsm_pool, \
         tc.tile_pool(name="otsb", bufs=1) as otsb_pool, \
         tc.tile_pool(name="ofull", bufs=1) as ofull_pool, \
         tc.tile_pool(name="ostage", bufs=2) as ostage_pool:

        a2a_in, a2a_out = io["a2a_in"], io["a2a_out"]

        blip_ps = ps_blip.tile([1, 128], F32, tag="blip", name="blip")

        def blip(n=1, gate=None):
            """Tiny scratch matmul(s): keep the PE HAM activity monitor busy.
            With gate=AP, the first blip waits for that tile (warm-up that
            starts exactly when a dependency lands)."""
            for i in range(n):
                lhs = gate if (gate is not None and i == 0) else ident_bf[0:1, 0:1]
                nc.tensor.matmul(blip_ps[0:1, 0:128], lhs,
                                 ones_row[:], start=True, stop=True,
                                 skip_group_check=True)

        # ---------------- weight / table loads (all async DMAs) -------------
        def weight_prep():
            # ordered by first use: q weights -> k -> v -> rope tables
            for nm in ("q", "k", "v"):
                nc.gpsimd.dma_start(wT[nm][:], io[f"w{nm}t"][:])
                nc.gpsimd.dma_start(l1[nm][:], io[f"l1{nm}"][:])
                nc.gpsimd.dma_start(l2T[nm][:], io[f"l2t{nm}"][:])
            nc.sync.dma_start(tri8T[:], io["tri8"][:])
            nc.sync.dma_start(wq_b_sb[:], io["wq_b"][:])
            # lora deltas for q/k/v: wT[:, c, :] += (l1[:,128c:+128]).T @ l2T
            for nm in ("q", "k", "v"):
                for c in range(NCHUNK):
                    dps = ps_sm.tile([128, 512], F32, tag="ps_sm", name="dps")
                    nc.tensor.matmul(dps[0:128, 0:128], l1[nm][:, 128 * c:128 * c + 128],
                                     l2T[nm][:], start=True, stop=True)
                    nc.vector.tensor_add(wT[nm][:, c, :], wT[nm][:, c, :],
                                         dps[0:128, 0:128])

        def wo_prep_gen():
            nc.gpsimd.dma_start(wo_b_sb[:], io["wo_b"][:])
            nc.gpsimd.dma_start(l1["o"][:], io["l1o"][:])
            nc.gpsimd.dma_start(l2To[:], io["l2to"][:])
            nc.gpsimd.dma_start(woT[:], io["wot"][:])
            yield
            for c in range(NCHUNK):
                dps = ps_big.tile([128, 1024], F32, tag="ps_big", name="wodps")
                for hh in range(2):
                    nc.tensor.matmul(dps[:, 512 * hh:512 * hh + 512],
                                     l1["o"][:, 128 * c:128 * c + 128],
                                     l2To[:, 512 * hh:512 * hh + 512],
                                     start=True, stop=True)
                nc.vector.tensor_add(woT[:, c, :], woT[:, c, :], dps[:])
                yield

        # ---------------- per-batch state ----------------
        qTs, kTs, Vxs, otAs, otBs, ofs = {}, {}, {}, {}, {}, {}
        xts_dbg = {}

        def proj_loads(h):
            """Issue the x^T chunk loads for quarter h (two half-chunks per
            contraction chunk so transfers pipeline with the previous
            quarter's compute)."""
            b, hh = h // 2, h % 2
            if hh == 0:
                qTs[b] = qkv_pool.tile([128, S], BF16, tag="qT", name="qT")
                kTs[b] = qkv_pool.tile([128, S], BF16, tag="kT", name="kT")
                Vxs[b] = qkv_pool.tile([128, NKT, 130], BF16, tag="Vx", name="Vx")
                nc.vector.memset(Vxs[b][:], 1.0)
            xt = {}
            for half in range(2):
                for c in range(NCHUNK):
                    xc = xt_pool.tile([128, 1024], BF16, tag="xt", name="xtc")
                    nc.gpsimd.dma_start(
                        xc[:], io["xT"][128 * c:128 * c + 128,
                                        QUARTER * h + 1024 * half:
                                        QUARTER * h + 1024 * half + 1024])
                    xt[(c, half)] = xc
            xts_dbg[h] = [xt[(0, 0)]]
            return xt

        def proj_mms_gen(h, xt):
            """Projection MMs + rope for quarter h. Yields after each
            per-tensor MM group so attention can interleave."""
            b, hh = h // 2, h % 2
            qT, kT, Vx = qTs[b], kTs[b], Vxs[b]
            for t in range(QUARTER // QT_TILE):
                s0 = QUARTER * hh + QT_TILE * t
                for nm in ("q", "k", "v"):
                    pp = ps_sm.tile([128, 512], F32, tag="ps_sm", name="pp")
                    for c in range(NCHUNK):
                        nc.tensor.matmul(pp[:], wT[nm][:, c, :],
                                         xt[(c, t // 2)][:, 512 * (t % 2):512 * (t % 2) + 512],
                                         start=(c == 0), stop=(c == NCHUNK - 1))
                    if nm == "v":
                        vst = rope_pool.tile([128, 512], BF16, tag="vstage")
                        nc.vector.tensor_copy(vst[:], pp[:])
                        if _dbg_all and h == 0 and t == 0:
                            nc.vector.tensor_copy(dbgC[:], vst[:])
                        for u in range(4):
                            kt = s0 // 128 + u
                            vps = ps_big.tile([128, 512], BF16, tag="ps_big", name="vps")
                            nc.tensor.transpose(vps[0:128, 0:128],
                                                vst[:, 128 * u:128 * u + 128],
                                                ident_bf[:])
                            nc.vector.tensor_copy(Vx[:, kt, 0:64], vps[0:128, 0:64])
                            nc.vector.tensor_copy(Vx[:, kt, 65:129], vps[0:128, 64:128])
                    else:
                        dstT = qT if nm == "q" else kT
                        cs = cosT4[:, s0:s0 + 512]
                        ss = sinT4[:, s0:s0 + 512]
                        t1 = rope_pool.tile([128, 512], BF16, tag="t1")
                        t2 = rope_pool.tile([128, 512], BF16, tag="t2")
                        if nm == "q":
                            nc.vector.scalar_tensor_tensor(
                                out=t1[:], in0=pp[:], scalar=wq_b_sb[:], in1=cs,
                                op0=ADD, op1=MULT)
                            nc.vector.scalar_tensor_tensor(
                                out=t2[:], in0=pp[:], scalar=wq_b_sb[:], in1=ss,
                                op0=ADD, op1=MULT)
                        else:
                            nc.vector.tensor_mul(t1[:], pp[:], cs)
                            nc.vector.tensor_mul(t2[:], pp[:], ss)
                        t2s = rope_pool.tile([128, 512], BF16, tag="t2s")
                        for (_o, _i) in ((0, 32), (32, 0), (64, 96), (96, 64)):
                            nc.sync.dma_start(t2s[_o:_o + 32, :], t2[_i:_i + 32, :])
                        if _dbg_all and h == 0 and t == 0 and nm == "q":
                            nc.vector.tensor_copy(dbgA[:], t1[:])
                            nc.vector.tensor_copy(dbgB[:], t2s[:])
                        nc.vector.tensor_add(dstT[:, s0:s0 + 512], t1[:], t2s[:])
                    yield

        # ---------------- filler queue shared across attention calls --------
        from collections import deque

        class Fillers:
            def __init__(self):
                self.q = deque()
                self.count = 0

            def add(self, *gens):
                self.q.extend(gens)

            def pump(self):
                while self.q:
                    try:
                        next(self.q[0])
                        self.count += 1
                        return True
                    except StopIteration:
                        self.q.popleft()
                return False

            def ensure(self, n):
                while self.count < n and self.pump():
                    pass

            def drain(self):
                while self.pump():
                    pass

        # ---------------- attention ----------------
        def attention_batch(b, j_lo, j_hi, fl, need=None):
            qT, kT, Vx = qTs[b], kTs[b], Vxs[b]
            if j_lo == 0:
                otAs[b] = otsb_pool.tile([64, S], BF16, tag="otA", name="otA")
                otBs[b] = otsb_pool.tile([64, S], BF16, tag="otB", name="otB")
            pairs_done = 0
            for j in range(j_lo, j_hi):
                if need is not None and j in need:
                    fl.ensure(need[j])
                q0 = QT_TILE * j
                otp = {}
                for hd_i in ("A", "B"):
                    otp[hd_i] = ps_ot.tile([65, 512], F32, tag="ot", name="otp")
                nkt = 4 * j + 4

                def emit_scores(p):
                    sps = {}
                    for hd_i in ("A", "B"):
                        sps[hd_i] = ps_big.tile([128, 1024], F32, tag="ps_big", name="sps")
                    for u in range(2):
                        i = 2 * p + u
                        n0 = max(0, 128 * (i - 4 * j))
                        for hd_i, base in (("A", 0), ("B", 64)):
                            nc.tensor.matmul(
                                sps[hd_i][:, 512 * u + n0:512 * u + 512],
                                kT[base:base + 64, 128 * i:128 * i + 128],
                                qT[base:base + 64, q0 + n0:q0 + 512],
                                start=True, stop=True,
                                tile_position=(base, 0))
                        if i - 4 * j >= 0:
                            cstar = i - 4 * j
                            for hd_i in ("A", "B"):
                                nc.vector.tensor_add(
                                    sps[hd_i][:, 512 * u + 128 * cstar:512 * u + 128 * cstar + 128],
                                    sps[hd_i][:, 512 * u + 128 * cstar:512 * u + 128 * cstar + 128],
                                    tri8T[:])
                    ptt = {}
                    for hd_i in ("A", "B"):
                        ptt[hd_i] = pt_pool.tile([128, 1024], BF16, tag="pt", name="ptt")
                        nc.scalar.activation(ptt[hd_i][:], sps[hd_i][:], EXP, scale=0.125)
                    return ptt

                def emit_pv(p, ptt):
                    for u in range(2):
                        i = 2 * p + u
                        n0 = max(0, 128 * (i - 4 * j))
                        for hd_i, vo in (("A", 0), ("B", 65)):
                            nc.tensor.matmul(
                                otp[hd_i][:, n0:512],
                                Vx[:, i, vo:vo + 65],
                                ptt[hd_i][:, 512 * u + n0:512 * u + 512],
                                start=(i == 0), stop=(i == nkt - 1),
                                skip_group_check=True)

                # software pipeline with lag 2: PV(p-2) streams right after
                # scores(p) with its exp long done, so the PE never waits
                pend = []
                for p in range(nkt // 2):
                    ptt = emit_scores(p)
                    # pump two pieces back-to-back: ~7us of uninterrupted PE
                    # work spans a full HAM window and un-throttles the clock
                    if fl.q and pairs_done % 3 == 0:
                        fl.pump()
                        fl.pump()
                    pairs_done += 1
                    pend.append((p, ptt))
                    if len(pend) > 2:
                        emit_pv(*pend.pop(0))
                for pp_ in pend:
                    emit_pv(*pp_)

                # normalize q-tile j and stage its a2a slice (dst core = j)
                for hd_i, dst in (("A", otAs[b]), ("B", otBs[b])):
                    stg = norm_pool.tile([65, 512], F32, tag="stg", name="stg")
                    nc.vector.tensor_copy(stg[:], otp[hd_i][:])
                    rz = normsm_pool.tile([1, 512], F32, tag="rz", name="rz")
                    nc.sync.dma_start(rz[:], stg[64:65, :])
                    rcp = normsm_pool.tile([1, 512], F32, tag="rcp", name="rcp")
                    nc.vector.reciprocal_approx_fast(rcp[:], rz[:])
                    rb = norm_pool.tile([64, 512], F32, tag="rb", name="rb")
                    nc.gpsimd.partition_broadcast(rb[:], rcp[:])
                    nc.vector.tensor_mul(dst[:, q0:q0 + 512], stg[0:64, :], rb[:])
                nc.sync.dma_start(a2a_in[b][j, 0:64, :], otAs[b][:, q0:q0 + 512])
                nc.sync.dma_start(a2a_in[b][j, 64:128, :], otBs[b][:, q0:q0 + 512])

        def a2a_start(b):
            nc.gpsimd.collective_compute(
                "AllToAll", mybir.AluOpType.bypass,
                replica_groups=[list(range(NCORES))],
                ins=[a2a_in[b].opt()], outs=[a2a_out[b].opt()])

        def oproj_gen(b, warm=False):
            ofs[b] = ofull_pool.tile([128, NCHUNK, 512], BF16, tag="ofull", name="of")
            nc.sync.dma_start(ofs[b][:], a2a_out[b][:].rearrange("c p f -> p c f"))
            if warm:
                # warm the PE exactly when the a2a lands: the gated blip burst
                # runs during the ofs load so the oproj matmuls start at K=8/8
                gate = normsm_pool.tile([1, 128], BF16, tag="ogate", name="ogate")
                nc.gpsimd.dma_start(gate[:], a2a_out[b][0, 0:1, 0:128])
                blip(40, gate=gate[0:1, 0:1])
            yield
            of = ofs[b]
            for t in range(4):
                for nn in range(2):
                    op = ps_sm.tile([128, 512], F32, tag="ps_sm", name="op")
                    for c in range(NCHUNK):
                        nc.tensor.matmul(op[:], of[:, c, 128 * t:128 * t + 128],
                                         woT[:, c, 512 * nn:512 * nn + 512],
                                         start=(c == 0), stop=False,
                                         skip_group_check=True)
                    nc.tensor.matmul(op[:], ones_row[:],
                                     wo_b_sb[:, 512 * nn:512 * nn + 512],
                                     start=False, stop=True, skip_group_check=True)
                    ost = ostage_pool.tile([128, 512], BF16, tag="ostage")
                    nc.vector.tensor_copy(ost[:], op[:])
                    nc.sync.dma_start(
                        io["out"][b, 128 * t:128 * t + 128, 512 * nn:512 * nn + 512],
                        ost[:])
                    yield

        def drain(g):
            for _ in g:
                pass

        # ---------------- phase program ----------------
        weight_prep()
        nc.gpsimd.dma_start(cosT4[:, 0:QUARTER], io["cost"][:, 0:QUARTER])
        nc.gpsimd.dma_start(sinT4[:, 0:QUARTER], io["sint"][:, 0:QUARTER])
        xt0 = proj_loads(0)
        nc.gpsimd.dma_start(cosT4[:, QUARTER:S], io["cost"][:, QUARTER:S])
        nc.gpsimd.dma_start(sinT4[:, QUARTER:S], io["sint"][:, QUARTER:S])
        if _ph >= 8:
            g0 = proj_mms_gen(0, xt0)
            for _ in range(6):          # t0, t1 of quarter 0
                next(g0)
            fl = Fillers()
            fl.add(g0)                  # 6 pieces left (t2, t3)
            attention_batch(0, 0, 2, fl)
            xt1 = proj_loads(1)
            fl.add(proj_mms_gen(1, xt1))
            attention_batch(0, 2, 4, fl, need={2: 3, 3: 6})
            xt2 = proj_loads(2)
            fl.add(proj_mms_gen(2, xt2), wo_prep_gen())
            attention_batch(0, 4, 8, fl, need={4: 9, 5: 12, 6: 15, 7: 18})
            a2a_start(0)
            xt3 = proj_loads(3)
            fl.add(proj_mms_gen(3, xt3))
            attention_batch(1, 0, 4, fl, need={0: 21, 1: 24, 2: 27, 3: 30})
            fl.add(oproj_gen(0))
            attention_batch(1, 4, 8, fl, need={4: 42, 5: 45, 6: 48, 7: 51})
            a2a_start(1)
            fl.drain()                  # oproj(0) remainder
            drain(oproj_gen(1, warm=True))
        else:
            fl = Fillers()
            if _ph >= 2: drain(proj_mms_gen(0, xt0))
            if _ph >= 3:
                xt1 = proj_loads(1)
                drain(proj_mms_gen(1, xt1))
            if _ph >= 4: attention_batch(0, 0, 8, fl)
            drain(wo_prep_gen())
            if _ph >= 6: a2a_start(0)
            if _ph >= 7:
                xt2 = proj_loads(2)
                drain(proj_mms_gen(2, xt2))
                xt3 = proj_loads(3)
                drain(proj_mms_gen(3, xt3))
                attention_batch(1, 0, 8, fl)
        if _ph < 8:
            dummy = ostage_pool.tile([128, 512], BF16, tag="ostage", name="dummy")
            nc.vector.memset(dummy[:], 0.0)
            nc.sync.dma_start(io["out"][0, 0:128, 0:512], dummy[:])

        _dbg = _os.environ.get("KDBG", "")
        if _dbg == "all":
            nc.gpsimd.dma_start(io["dbg"][:, 0:2048], qTs[0][:, 0:2048])
            nc.gpsimd.dma_start(io["dbg"][:, 2048:2048 + 16 * 130],
                                Vxs[0][:, 0:16, :].rearrange("p k f -> p (k f)"))
            nc.gpsimd.dma_start(io["dbg"][:, 4128:5152],
                                wT["q"][:].rearrange("p c f -> p (c f)"))
            if 0 in xts_dbg:
                nc.gpsimd.dma_start(io["dbg"][:, 5152:6176], xts_dbg[0][0][:, 0:1024])
            nc.gpsimd.dma_start(io["dbg"][:, 6176:6688], dbgA[:])
            nc.gpsimd.dma_start(io["dbg"][:, 6688:7200], dbgB[:])
            nc.gpsimd.dma_start(io["dbg"][:, 7200:7712], dbgC[:])
        elif _dbg == "qT":
            nc.gpsimd.dma_start(io["dbg"][:, 0:4096], qTs[0][:])
        elif _dbg == "kT":
            nc.gpsimd.dma_start(io["dbg"][:, 0:4096], kTs[0][:])
        elif _dbg == "Vx":
            nc.gpsimd.dma_start(io["dbg"][:, 0:NKT * 130], Vxs[0][:])
        elif _dbg == "otA":
            nc.gpsimd.dma_start(io["dbg"][0:64, 0:4096], otAs[0][:])
            nc.gpsimd.dma_start(io["dbg"][64:128, 0:4096], otBs[0][:])
        elif _dbg == "wqT":
            nc.gpsimd.dma_start(io["dbg"][:, 0:1024], wT["q"][:].rearrange("p c f -> p (c f)"))
        else:
            dz = ostage_pool.tile([128, 512], F32, tag="ostage", name="dz")
            nc.vector.memset(dz[:], 0.0)
            nc.sync.dma_start(io["dbg"][:, 0:512], dz[:])
    persist_ctx.__exit__(None, None, None)


def _build():
    nc = bacc.Bacc("TRN2", target_bir_lowering=False, debug=False,
                   num_devices=NCORES)
    io = {}
    dram_in = lambda name, shape: nc.dram_tensor(name, shape, F32, kind="ExternalInput").ap()
    io["xT"] = dram_in("xT", [D, TOK])
    io["tri8"] = dram_in("tri8", [128, 128])
    io["cost"] = dram_in("cost", [128, S])
    io["sint"] = dram_in("sint", [128, S])
    io["wqt"] = dram_in("wqt", [128, NCHUNK, 128])
    io["wkt"] = dram_in("wkt", [128, NCHUNK, 128])
    io["wvt"] = dram_in("wvt", [128, NCHUNK, 128])
    io["wot"] = dram_in("wot", [128, NCHUNK, 1024])
    io["wq_b"] = dram_in("wq_b", [128, 1])
    io["wo_b"] = dram_in("wo_b", [1, D])
    for nm in ("q", "k", "v", "o"):
        io[f"l1{nm}"] = dram_in(f"l1{nm}", [R, D])
    for nm in ("q", "k", "v"):
        io[f"l2t{nm}"] = dram_in(f"l2t{nm}", [R, 128])
    io["l2to"] = dram_in("l2to", [R, D])
    io["out"] = nc.dram_tensor("out", [B, 512, D], BF16, kind="ExternalOutput").ap()
    io["dbg"] = nc.dram_tensor("dbg", [128, 8192], F32, kind="ExternalOutput").ap()

    with tile.TileContext(nc) as tc:
        with tc.tile_pool(name="dram", bufs=1, space="DRAM") as dram:
            io["a2a_in"] = [dram.tile([NCORES, 128, 512], BF16, name=f"a2ai{b}") for b in range(B)]
            io["a2a_out"] = [dram.tile([NCORES, 128, 512], BF16, name=f"a2ao{b}") for b in range(B)]
            _emit(nc, tc, io)
    nc.compile()
    return nc


def _shard_inputs(inputs):
    f = lambda a: np.ascontiguousarray(np.asarray(a, dtype=np.float32))
    x = f(inputs["x"]).reshape(TOK, D)
    xT = np.ascontiguousarray(x.T)                       # [D, TOK]
    mask = f(inputs["mask"]).reshape(S, S)
    tri8 = np.ascontiguousarray(8.0 * mask[:128, :128].T)
    cos, sin = f(inputs["freqs_cos"]), f(inputs["freqs_sin"])
    cost = np.ascontiguousarray(np.tile(cos.T, (4, 1)))  # [128, S]
    st = sin.T
    sint = np.ascontiguousarray(np.concatenate([st, -st, st, -st], axis=0))
    wq, wk, wv, wo = f(inputs["wq_w"]), f(inputs["wk_w"]), f(inputs["wv_w"]), f(inputs["wo_w"])
    wq_b, wo_b = f(inputs["wq_b"]), f(inputs["wo_b"])
    l1 = {nm: f(inputs[f"lora_{nm}_l1"]) for nm in ("q", "k", "v", "o")}
    l2 = {nm: f(inputs[f"lora_{nm}_l2"]) for nm in ("q", "k", "v", "o")}

    def wt_chunked(w_rows):
        # [128 out, 1024 in] -> W^T [1024, 128] -> [128 part, 8 chunk, 128 out]
        t = w_rows.T.reshape(NCHUNK, 128, 128).transpose(1, 0, 2)
        return np.ascontiguousarray(t)

    wot = np.ascontiguousarray(wo.T.reshape(NCHUNK, 128, 1024).transpose(1, 0, 2))

    perm64 = np.concatenate([np.arange(0, 64, 2), np.arange(1, 64, 2)])
    in_maps = []
    for c in range(NCORES):
        rows_p = np.concatenate([128 * c + perm64, 128 * c + 64 + perm64])
        rows_n = np.arange(128 * c, 128 * c + 128)
        m = {
            "xT": xT,
            "tri8": tri8,
            "cost": cost, "sint": sint,
            "wqt": wt_chunked(wq[rows_p]),
            "wkt": wt_chunked(wk[rows_p]),
            "wvt": wt_chunked(wv[rows_n]),
            "wot": wot,
            "wq_b": np.ascontiguousarray(wq_b[rows_p]).reshape(128, 1),
            "wo_b": wo_b.reshape(1, D),
            "l2tq": np.ascontiguousarray(l2["q"][rows_p].T),
            "l2tk": np.ascontiguousarray(l2["k"][rows_p].T),
            "l2tv": np.ascontiguousarray(l2["v"][rows_n].T),
            "l2to": np.ascontiguousarray(l2["o"].T),
        }
        for nm in ("q", "k", "v", "o"):
            m[f"l1{nm}"] = l1[nm]
        in_maps.append(m)
    return in_maps


def _enable_ldw_opt():
    import concourse.bass_utils as _bu
    if getattr(_bu, "_ldw_patched", False):
        return
    _orig = _bu.run_command
    def _patched(argv, **kw):
        argv = ["--enable-ldw-opt=true" if a == "--enable-ldw-opt=false" else a
                for a in argv]
        return _orig(argv, **kw)
    _bu.run_command = _patched
    _bu._ldw_patched = True


def _install_trace_hook():
    """Provide antenv.axon_hooks (absent in this image) so trace=True works."""
    import types
    try:
        import antenv.axon_hooks  # noqa
        return
    except ImportError:
        pass
    try:
        from trn_agent_boot.trn_boot import _ntff_profile_via_ctypes
        hook = _ntff_profile_via_ctypes("/opt/axon/libaxon_pjrt.so")
        mod = types.ModuleType("antenv.axon_hooks")
        mod.get_axon_ntff_profile_hook = lambda: hook
        mod.set_axon_ntff_profile_hook = lambda h: None
        sys.modules["antenv.axon_hooks"] = mod
        import concourse.bass_utils as _bu
        _bu.upload_artifacts = lambda d: str(d)
    except Exception as e:
        print(f"trace hook install failed: {e}")


def kernel(**inputs):
    global LAST_EXEC_NS
    import os as _os
    if _os.environ.get("KLDW"):
        _enable_ldw_opt()
    if "nc" not in _CACHE:
        _CACHE["nc"] = _build()
    nc = _CACHE["nc"]
    in_maps = _shard_inputs(inputs)
    if TRACE:
        _install_trace_hook()
    res = run_bass_kernel_spmd(nc, in_maps, core_ids=list(range(NCORES)),
                               trace=TRACE)
    _CACHE["res"] = res
    LAST_EXEC_NS = res.exec_time_ns
    out = np.empty((B, S, D), dtype=np.float32)
    for c in range(NCORES):
        out[:, 512 * c:512 * (c + 1), :] = np.asarray(
            res.results[c]["out"], dtype=np.float32)
    return out


# revision 43
# speedup vs baseline: 1.1746x; 1.1746x over previous
"""Trainium2 Bass kernel for nn_Attention_40690520163106.

Multi-head causal attention with RoPE + LoRA on 8 NeuronCores.
Sharding: tensor-parallel over the 16 heads (2 heads/core), data-replicated
over batch; AllToAll reshard before the output projection so each core
computes a disjoint token slice of the final output (no reduction needed).

v2: host-side layout transforms (transposed x/weights, prebuilt rope tables),
SWDGE cast-loads, fast softmax normalize, and projection/output-projection
work interleaved into the attention loop so the PE array never idles.

Self-contained: hardcodes all shapes; reads nothing from /root/problem.
"""

import sys
import numpy as np

for _p in ("/opt/trn_rl_repo", "/root/.axon_site/_ro/trn_rl_repo"):
    if _p not in sys.path:
        sys.path.insert(0, _p)

import concourse.bass as bass
import concourse.mybir as mybir
import concourse.tile as tile
from concourse import bacc
from concourse.bass_utils import run_bass_kernel_spmd
from concourse.masks import make_identity

F32 = mybir.dt.float32
BF16 = mybir.dt.bfloat16
EXP = mybir.ActivationFunctionType.Exp
ADD = mybir.AluOpType.add
MULT = mybir.AluOpType.mult

B, S, D, H, HD, R = 2, 4096, 1024, 16, 64, 16
NCORES = 8
TOK = B * S
QT_TILE = 512               # q free-dim tile (one psum bank of fp32)
NQ = S // QT_TILE           # 8 q-tiles per batch
NKT = S // 128              # 32 k-tiles per batch
QUARTER = 2048              # tokens per projection quarter
NCHUNK = D // 128           # 8 contraction chunks

TRACE = False
LAST_EXEC_NS = None

_CACHE = {}


def _emit(nc, tc, io):
    import os as _os
    _ph = int(_os.environ.get("KPH", "99"))
    _dbg_all = _os.environ.get("KDBG", "") == "all"

    persist_ctx = tc.tile_pool(name="persist", bufs=1)
    persist_pool = persist_ctx.__enter__()
    sb1 = lambda shape, dt, name: persist_pool.tile(shape, dt, name=name, tag=name)

    # ---------------- persistent SBUF tensors ----------------
    ident_bf = sb1([128, 128], BF16, "ident_bf")
    make_identity(nc, ident_bf[:])

    wT = {nm: sb1([128, NCHUNK, 128], BF16, f"w{nm}T") for nm in ("q", "k", "v")}
    woT = sb1([128, NCHUNK, 1024], BF16, "woT")
    wq_b_sb = sb1([128, 1], F32, "wq_b_sb")
    wo_b_sb = sb1([1, 1024], BF16, "wo_b_sb")
    ones_row = sb1([1, 128], BF16, "ones_row")
    nc.vector.memset(ones_row[:], 1.0)
    tri8T = sb1([128, 128], F32, "tri8T")
    cosT4 = sb1([128, S], BF16, "cosT4")
    sinT4 = sb1([128, S], BF16, "sinT4")
    dbgA = sb1([128, 512], F32, "dbgA")
    dbgB = sb1([128, 512], F32, "dbgB")
    dbgC = sb1([128, 512], F32, "dbgC")
    l1 = {nm: sb1([16, 1024], BF16, f"l1{nm}") for nm in ("q", "k", "v", "o")}
    l2T = {nm: sb1([16, 128], BF16, f"l2T{nm}") for nm in ("q", "k", "v")}
    l2To = sb1([16, 1024], BF16, "l2To")

    with tc.tile_pool(name="ps_sm", bufs=1, space="PSUM") as ps_sm, \
         tc.tile_pool(name="ps_blip", bufs=1, space="PSUM") as ps_blip, \
         tc.tile_pool(name="ps_big", bufs=2, space="PSUM") as ps_big, \
         tc.tile_pool(name="ps_ot", bufs=2, space="PSUM") as ps_ot, \
         tc.tile_pool(name="xt", bufs=16) as xt_pool, \
         tc.tile_pool(name="qkv", bufs=2) as qkv_pool, \
         tc.tile_pool(name="rope", bufs=2) as rope_pool, \
         tc.tile_pool(name="pt", bufs=6) as pt_pool, \
         tc.tile_pool(name="norm", bufs=2) as norm_pool, \
         tc.tile_pool(name="normsm", bufs=3) as normsm_pool, \
         tc.tile_pool(name="otsb", bufs=1) as otsb_pool, \
         tc.tile_pool(name="ofull", bufs=1) as ofull_pool, \
         tc.tile_pool(name="ostage", bufs=2) as ostage_pool:

        a2a_in, a2a_out = io["a2a_in"], io["a2a_out"]

        blip_ps = ps_blip.tile([1, 128], F32, tag="blip", name="blip")

        def blip(n=1, gate=None):
            """Tiny scratch matmul(s): keep the PE HAM activity monitor busy.
            With gate=AP, the first blip waits for that tile (warm-up that
            starts exactly when a dependency lands)."""
            for i in range(n):
                lhs = gate if (gate is not None and i == 0) else ident_bf[0:1, 0:1]
                nc.tensor.matmul(blip_ps[0:1, 0:128], lhs,
                                 ones_row[:], start=True, stop=True,
                                 skip_group_check=True)

        # ---------------- weight / table loads (all async DMAs) -------------
        def weight_prep():
            # ordered by first use: q weights -> k -> v -> rope tables
            for nm in ("q", "k", "v"):
                nc.gpsimd.dma_start(wT[nm][:], io[f"w{nm}t"][:])
                nc.gpsimd.dma_start(l1[nm][:], io[f"l1{nm}"][:])
                nc.gpsimd.dma_start(l2T[nm][:], io[f"l2t{nm}"][:])
            nc.sync.dma_start(tri8T[:], io["tri8"][:])
            nc.sync.dma_start(wq_b_sb[:], io["wq_b"][:])
            # lora deltas for q/k/v: wT[:, c, :] += (l1[:,128c:+128]).T @ l2T
            for nm in ("q", "k", "v"):
                for c in range(NCHUNK):
                    dps = ps_sm.tile([128, 512], F32, tag="ps_sm", name="dps")
                    nc.tensor.matmul(dps[0:128, 0:128], l1[nm][:, 128 * c:128 * c + 128],
                                     l2T[nm][:], start=True, stop=True)
                    nc.vector.tensor_add(wT[nm][:, c, :], wT[nm][:, c, :],
                                         dps[0:128, 0:128])

        def wo_prep_gen():
            nc.gpsimd.dma_start(wo_b_sb[:], io["wo_b"][:])
            nc.gpsimd.dma_start(l1["o"][:], io["l1o"][:])
            nc.gpsimd.dma_start(l2To[:], io["l2to"][:])
            nc.gpsimd.dma_start(woT[:], io["wot"][:])
            yield
            for c in range(NCHUNK):
                dps = ps_big.tile([128, 1024], F32, tag="ps_big", name="wodps")
                for hh in range(2):
                    nc.tensor.matmul(dps[:, 512 * hh:512 * hh + 512],
                                     l1["o"][:, 128 * c:128 * c + 128],
                                     l2To[:, 512 * hh:512 * hh + 512],
                                     start=True, stop=True)
                nc.vector.tensor_add(woT[:, c, :], woT[:, c, :], dps[:])
                yield

        # ---------------- per-batch state ----------------
        qTs, kTs, Vxs, otAs, otBs, ofs = {}, {}, {}, {}, {}, {}
        xts_dbg = {}

        def proj_loads(h):
            """Issue the x^T chunk loads for quarter h (two half-chunks per
            contraction chunk so transfers pipeline with the previous
            quarter's compute)."""
            b, hh = h // 2, h % 2
            if hh == 0:
                qTs[b] = qkv_pool.tile([128, S], BF16, tag="qT", name="qT")
                kTs[b] = qkv_pool.tile([128, S], BF16, tag="kT", name="kT")
                Vxs[b] = qkv_pool.tile([128, NKT, 130], BF16, tag="Vx", name="Vx")
                nc.vector.memset(Vxs[b][:], 1.0)
            xt = {}
            for half in range(2):
                for c in range(NCHUNK):
                    xc = xt_pool.tile([128, 1024], BF16, tag="xt", name="xtc")
                    nc.gpsimd.dma_start(
                        xc[:], io["xT"][128 * c:128 * c + 128,
                                        QUARTER * h + 1024 * half:
                                        QUARTER * h + 1024 * half + 1024])
                    xt[(c, half)] = xc
            xts_dbg[h] = [xt[(0, 0)]]
            return xt

        def proj_mms_gen(h, xt):
            """Projection MMs + rope for quarter h. Yields after each
            per-tensor MM group so attention can interleave."""
            b, hh = h // 2, h % 2
            qT, kT, Vx = qTs[b], kTs[b], Vxs[b]
            for t in range(QUARTER // QT_TILE):
                s0 = QUARTER * hh + QT_TILE * t
                for nm in ("q", "k", "v"):
                    pp = ps_sm.tile([128, 512], F32, tag="ps_sm", name="pp")
                    for c in range(NCHUNK):
                        nc.tensor.matmul(pp[:], wT[nm][:, c, :],
                                         xt[(c, t // 2)][:, 512 * (t % 2):512 * (t % 2) + 512],
                                         start=(c == 0), stop=(c == NCHUNK - 1))
                    if nm == "v":
                        vst = rope_pool.tile([128, 512], BF16, tag="vstage")
                        nc.vector.tensor_copy(vst[:], pp[:])
                        if _dbg_all and h == 0 and t == 0:
                            nc.vector.tensor_copy(dbgC[:], vst[:])
                        for u in range(4):
                            kt = s0 // 128 + u
                            vps = ps_big.tile([128, 512], BF16, tag="ps_big", name="vps")
                            nc.tensor.transpose(vps[0:128, 0:128],
                                                vst[:, 128 * u:128 * u + 128],
                                                ident_bf[:])
                            nc.vector.tensor_copy(Vx[:, kt, 0:64], vps[0:128, 0:64])
                            nc.vector.tensor_copy(Vx[:, kt, 65:129], vps[0:128, 64:128])
                    else:
                        dstT = qT if nm == "q" else kT
                        cs = cosT4[:, s0:s0 + 512]
                        ss = sinT4[:, s0:s0 + 512]
                        t1 = rope_pool.tile([128, 512], BF16, tag="t1")
                        t2 = rope_pool.tile([128, 512], BF16, tag="t2")
                        if nm == "q":
                            nc.vector.scalar_tensor_tensor(
                                out=t1[:], in0=pp[:], scalar=wq_b_sb[:], in1=cs,
                                op0=ADD, op1=MULT)
                            nc.vector.scalar_tensor_tensor(
                                out=t2[:], in0=pp[:], scalar=wq_b_sb[:], in1=ss,
                                op0=ADD, op1=MULT)
                        else:
                            nc.vector.tensor_mul(t1[:], pp[:], cs)
                            nc.vector.tensor_mul(t2[:], pp[:], ss)
                        t2s = rope_pool.tile([128, 512], BF16, tag="t2s")
                        for (_o, _i) in ((0, 32), (32, 0), (64, 96), (96, 64)):
                            nc.sync.dma_start(t2s[_o:_o + 32, :], t2[_i:_i + 32, :])
                        if _dbg_all and h == 0 and t == 0 and nm == "q":
                            nc.vector.tensor_copy(dbgA[:], t1[:])
                            nc.vector.tensor_copy(dbgB[:], t2s[:])
                        nc.vector.tensor_add(dstT[:, s0:s0 + 512], t1[:], t2s[:])
                    yield

        # ---------------- filler queue shared across attention calls --------
        from collections import deque

        class Fillers:
            def __init__(self):
                self.q = deque()
                self.count = 0

            def add(self, *gens):
                self.q.extend(gens)

            def pump(self):
                while self.q:
                    try:
                        next(self.q[0])
                        self.count += 1
                        return True
                    except StopIteration:
                        self.q.popleft()
                return False

            def ensure(self, n):
                while self.count < n and self.pump():
                    pass

            def drain(self):
                while self.pump():
                    pass

        # ---------------- attention ----------------
        def attention_batch(b, j_lo, j_hi, fl, need=None):
            qT, kT, Vx = qTs[b], kTs[b], Vxs[b]
            if j_lo == 0:
                otAs[b] = otsb_pool.tile([64, S], BF16, tag="otA", name="otA")
                otBs[b] = otsb_pool.tile([64, S], BF16, tag="otB", name="otB")
            pairs_done = 0
            for j in range(j_lo, j_hi):
                if need is not None and j in need:
                    fl.ensure(need[j])
                q0 = QT_TILE * j
                otp = {}
                for hd_i in ("A", "B"):
                    otp[hd_i] = ps_ot.tile([65, 512], F32, tag="ot", name="otp")
                nkt = 4 * j + 4

                def emit_scores(p):
                    sps = {}
                    for hd_i in ("A", "B"):
                        sps[hd_i] = ps_big.tile([128, 1024], F32, tag="ps_big", name="sps")
                    for u in range(2):
                        i = 2 * p + u
                        n0 = max(0, 128 * (i - 4 * j))
                        for hd_i, base in (("A", 0), ("B", 64)):
                            nc.tensor.matmul(
                                sps[hd_i][:, 512 * u + n0:512 * u + 512],
                                kT[base:base + 64, 128 * i:128 * i + 128],
                                qT[base:base + 64, q0 + n0:q0 + 512],
                                start=True, stop=True,
                                tile_position=(base, 0))
                        if i - 4 * j >= 0:
                            cstar = i - 4 * j
                            for hd_i in ("A", "B"):
                                nc.vector.tensor_add(
                                    sps[hd_i][:, 512 * u + 128 * cstar:512 * u + 128 * cstar + 128],
                                    sps[hd_i][:, 512 * u + 128 * cstar:512 * u + 128 * cstar + 128],
                                    tri8T[:])
                    ptt = {}
                    for hd_i in ("A", "B"):
                        ptt[hd_i] = pt_pool.tile([128, 1024], BF16, tag="pt", name="ptt")
                        nc.scalar.activation(ptt[hd_i][:], sps[hd_i][:], EXP, scale=0.125)
                    return ptt

                def emit_pv(p, ptt):
                    for u in range(2):
                        i = 2 * p + u
                        n0 = max(0, 128 * (i - 4 * j))
                        for hd_i, vo in (("A", 0), ("B", 65)):
                            nc.tensor.matmul(
                                otp[hd_i][:, n0:512],
                                Vx[:, i, vo:vo + 65],
                                ptt[hd_i][:, 512 * u + n0:512 * u + 512],
                                start=(i == 0), stop=(i == nkt - 1),
                                skip_group_check=True)

                # software pipeline with lag 2: PV(p-2) streams right after
                # scores(p) with its exp long done, so the PE never waits
                pend = []
                for p in range(nkt // 2):
                    ptt = emit_scores(p)
                    if fl.q and (2 * pairs_done) % 3 != 2:
                        fl.pump()
                    pairs_done += 1
                    pend.append((p, ptt))
                    if len(pend) > 2:
                        emit_pv(*pend.pop(0))
                for pp_ in pend:
                    emit_pv(*pp_)

                # normalize q-tile j and stage its a2a slice (dst core = j)
                for hd_i, dst in (("A", otAs[b]), ("B", otBs[b])):
                    stg = norm_pool.tile([65, 512], F32, tag="stg", name="stg")
                    nc.vector.tensor_copy(stg[:], otp[hd_i][:])
                    rz = normsm_pool.tile([1, 512], F32, tag="rz", name="rz")
                    nc.sync.dma_start(rz[:], stg[64:65, :])
                    rcp = normsm_pool.tile([1, 512], F32, tag="rcp", name="rcp")
                    nc.vector.reciprocal_approx_fast(rcp[:], rz[:])
                    rb = norm_pool.tile([64, 512], F32, tag="rb", name="rb")
                    nc.gpsimd.partition_broadcast(rb[:], rcp[:])
                    nc.vector.tensor_mul(dst[:, q0:q0 + 512], stg[0:64, :], rb[:])
                nc.sync.dma_start(a2a_in[b][j, 0:64, :], otAs[b][:, q0:q0 + 512])
                nc.sync.dma_start(a2a_in[b][j, 64:128, :], otBs[b][:, q0:q0 + 512])

        def a2a_start(b):
            nc.gpsimd.collective_compute(
                "AllToAll", mybir.AluOpType.bypass,
                replica_groups=[list(range(NCORES))],
                ins=[a2a_in[b].opt()], outs=[a2a_out[b].opt()])

        def oproj_gen(b, warm=False):
            ofs[b] = ofull_pool.tile([128, NCHUNK, 512], BF16, tag="ofull", name="of")
            nc.sync.dma_start(ofs[b][:], a2a_out[b][:].rearrange("c p f -> p c f"))
            if warm:
                # warm the PE exactly when the a2a lands: the gated blip burst
                # runs during the ofs load so the oproj matmuls start at K=8/8
                gate = normsm_pool.tile([1, 128], BF16, tag="ogate", name="ogate")
                nc.gpsimd.dma_start(gate[:], a2a_out[b][0, 0:1, 0:128])
                blip(40, gate=gate[0:1, 0:1])
            yield
            of = ofs[b]
            for t in range(4):
                for nn in range(2):
                    op = ps_sm.tile([128, 512], F32, tag="ps_sm", name="op")
                    for c in range(NCHUNK):
                        nc.tensor.matmul(op[:], of[:, c, 128 * t:128 * t + 128],
                                         woT[:, c, 512 * nn:512 * nn + 512],
                                         start=(c == 0), stop=False,
                                         skip_group_check=True)
                    nc.tensor.matmul(op[:], ones_row[:],
                                     wo_b_sb[:, 512 * nn:512 * nn + 512],
                                     start=False, stop=True, skip_group_check=True)
                    ost = ostage_pool.tile([128, 512], BF16, tag="ostage")
                    nc.vector.tensor_copy(ost[:], op[:])
                    nc.sync.dma_start(
                        io["out"][b, 128 * t:128 * t + 128, 512 * nn:512 * nn + 512],
                        ost[:])
                    yield

        def drain(g):
            for _ in g:
                pass

        # ---------------- phase program ----------------
        weight_prep()
        nc.gpsimd.dma_start(cosT4[:, 0:QUARTER], io["cost"][:, 0:QUARTER])
        nc.gpsimd.dma_start(sinT4[:, 0:QUARTER], io["sint"][:, 0:QUARTER])
        xt0 = proj_loads(0)
        nc.gpsimd.dma_start(cosT4[:, QUARTER:S], io["cost"][:, QUARTER:S])
        nc.gpsimd.dma_start(sinT4[:, QUARTER:S], io["sint"][:, QUARTER:S])
        if _ph >= 8:
            g0 = proj_mms_gen(0, xt0)
            for _ in range(6):          # t0, t1 of quarter 0
                next(g0)
            fl = Fillers()
            fl.add(g0)                  # 6 pieces left (t2, t3)
            attention_batch(0, 0, 2, fl)
            xt1 = proj_loads(1)
            fl.add(proj_mms_gen(1, xt1))
            attention_batch(0, 2, 4, fl, need={2: 3, 3: 6})
            xt2 = proj_loads(2)
            fl.add(proj_mms_gen(2, xt2), wo_prep_gen())
            attention_batch(0, 4, 8, fl, need={4: 9, 5: 12, 6: 15, 7: 18})
            a2a_start(0)
            xt3 = proj_loads(3)
            fl.add(proj_mms_gen(3, xt3))
            attention_batch(1, 0, 4, fl, need={0: 21, 1: 24, 2: 27, 3: 30})
            fl.add(oproj_gen(0))
            attention_batch(1, 4, 8, fl, need={4: 42, 5: 45, 6: 48, 7: 51})
            a2a_start(1)
            fl.drain()                  # oproj(0) remainder
            drain(oproj_gen(1, warm=True))
        else:
            fl = Fillers()
            if _ph >= 2: drain(proj_mms_gen(0, xt0))
            if _ph >= 3:
                xt1 = proj_loads(1)
                drain(proj_mms_gen(1, xt1))
            if _ph >= 4: attention_batch(0, 0, 8, fl)
            drain(wo_prep_gen())
            if _ph >= 6: a2a_start(0)
            if _ph >= 7:
                xt2 = proj_loads(2)
                drain(proj_mms_gen(2, xt2))
                xt3 = proj_loads(3)
                drain(proj_mms_gen(3, xt3))
                attention_batch(1, 0, 8, fl)
        if _ph < 8:
            dummy = ostage_pool.tile([128, 512], BF16, tag="ostage", name="dummy")
            nc.vector.memset(dummy[:], 0.0)
            nc.sync.dma_start(io["out"][0, 0:128, 0:512], dummy[:])

        _dbg = _os.environ.get("KDBG", "")
        if _dbg == "all":
            nc.gpsimd.dma_start(io["dbg"][:, 0:2048], qTs[0][:, 0:2048])
            nc.gpsimd.dma_start(io["dbg"][:, 2048:2048 + 16 * 130],
                                Vxs[0][:, 0:16, :].rearrange("p k f -> p (k f)"))
            nc.gpsimd.dma_start(io["dbg"][:, 4128:5152],
                                wT["q"][:].rearrange("p c f -> p (c f)"))
            if 0 in xts_dbg:
                nc.gpsimd.dma_start(io["dbg"][:, 5152:6176], xts_dbg[0][0][:, 0:1024])
            nc.gpsimd.dma_start(io["dbg"][:, 6176:6688], dbgA[:])
            nc.gpsimd.dma_start(io["dbg"][:, 6688:7200], dbgB[:])
            nc.gpsimd.dma_start(io["dbg"][:, 7200:7712], dbgC[:])
        elif _dbg == "qT":
            nc.gpsimd.dma_start(io["dbg"][:, 0:4096], qTs[0][:])
        elif _dbg == "kT":
            nc.gpsimd.dma_start(io["dbg"][:, 0:4096], kTs[0][:])
        elif _dbg == "Vx":
            nc.gpsimd.dma_start(io["dbg"][:, 0:NKT * 130], Vxs[0][:])
        elif _dbg == "otA":
            nc.gpsimd.dma_start(io["dbg"][0:64, 0:4096], otAs[0][:])
            nc.gpsimd.dma_start(io["dbg"][64:128, 0:4096], otBs[0][:])
        elif _dbg == "wqT":
            nc.gpsimd.dma_start(io["dbg"][:, 0:1024], wT["q"][:].rearrange("p c f -> p (c f)"))
        else:
            dz = ostage_pool.tile([128, 512], F32, tag="ostage", name="dz")
            nc.vector.memset(dz[:], 0.0)
            nc.sync.dma_start(io["dbg"][:, 0:512], dz[:])
    persist_ctx.__exit__(None, None, None)


def _build():
    nc = bacc.Bacc("TRN2", target_bir_lowering=False, debug=False,
                   num_devices=NCORES)
    io = {}
    dram_in = lambda name, shape: nc.dram_tensor(name, shape, F32, kind="ExternalInput").ap()
    io["xT"] = dram_in("xT", [D, TOK])
    io["tri8"] = dram_in("tri8", [128, 128])
    io["cost"] = dram_in("cost", [128, S])
    io["sint"] = dram_in("sint", [128, S])
    io["wqt"] = dram_in("wqt", [128, NCHUNK, 128])
    io["wkt"] = dram_in("wkt", [128, NCHUNK, 128])
    io["wvt"] = dram_in("wvt", [128, NCHUNK, 128])
    io["wot"] = dram_in("wot", [128, NCHUNK, 1024])
    io["wq_b"] = dram_in("wq_b", [128, 1])
    io["wo_b"] = dram_in("wo_b", [1, D])
    for nm in ("q", "k", "v", "o"):
        io[f"l1{nm}"] = dram_in(f"l1{nm}", [R, D])
    for nm in ("q", "k", "v"):
        io[f"l2t{nm}"] = dram_in(f"l2t{nm}", [R, 128])
    io["l2to"] = dram_in("l2to", [R, D])
    io["out"] = nc.dram_tensor("out", [B, 512, D], BF16, kind="ExternalOutput").ap()
    io["dbg"] = nc.dram_tensor("dbg", [128, 8192], F32, kind="ExternalOutput").ap()

    with tile.TileContext(nc) as tc:
        with tc.tile_pool(name="dram", bufs=1, space="DRAM") as dram:
            io["a2a_in"] = [dram.tile([NCORES, 128, 512], BF16, name=f"a2ai{b}") for b in range(B)]
            io["a2a_out"] = [dram.tile([NCORES, 128, 512], BF16, name=f"a2ao{b}") for b in range(B)]
            _emit(nc, tc, io)
    nc.compile()
    return nc


def _shard_inputs(inputs):
    f = lambda a: np.ascontiguousarray(np.asarray(a, dtype=np.float32))
    x = f(inputs["x"]).reshape(TOK, D)
    xT = np.ascontiguousarray(x.T)                       # [D, TOK]
    mask = f(inputs["mask"]).reshape(S, S)
    tri8 = np.ascontiguousarray(8.0 * mask[:128, :128].T)
    cos, sin = f(inputs["freqs_cos"]), f(inputs["freqs_sin"])
    cost = np.ascontiguousarray(np.tile(cos.T, (4, 1)))  # [128, S]
    st = sin.T
    sint = np.ascontiguousarray(np.concatenate([st, -st, st, -st], axis=0))
    wq, wk, wv, wo = f(inputs["wq_w"]), f(inputs["wk_w"]), f(inputs["wv_w"]), f(inputs["wo_w"])
    wq_b, wo_b = f(inputs["wq_b"]), f(inputs["wo_b"])
    l1 = {nm: f(inputs[f"lora_{nm}_l1"]) for nm in ("q", "k", "v", "o")}
    l2 = {nm: f(inputs[f"lora_{nm}_l2"]) for nm in ("q", "k", "v", "o")}

    def wt_chunked(w_rows):
        # [128 out, 1024 in] -> W^T [1024, 128] -> [128 part, 8 chunk, 128 out]
        t = w_rows.T.reshape(NCHUNK, 128, 128).transpose(1, 0, 2)
        return np.ascontiguousarray(t)

    wot = np.ascontiguousarray(wo.T.reshape(NCHUNK, 128, 1024).transpose(1, 0, 2))

    perm64 = np.concatenate([np.arange(0, 64, 2), np.arange(1, 64, 2)])
    in_maps = []
    for c in range(NCORES):
        rows_p = np.concatenate([128 * c + perm64, 128 * c + 64 + perm64])
        rows_n = np.arange(128 * c, 128 * c + 128)
        m = {
            "xT": xT,
            "tri8": tri8,
            "cost": cost, "sint": sint,
            "wqt": wt_chunked(wq[rows_p]),
            "wkt": wt_chunked(wk[rows_p]),
            "wvt": wt_chunked(wv[rows_n]),
            "wot": wot,
            "wq_b": np.ascontiguousarray(wq_b[rows_p]).reshape(128, 1),
            "wo_b": wo_b.reshape(1, D),
            "l2tq": np.ascontiguousarray(l2["q"][rows_p].T),
            "l2tk": np.ascontiguousarray(l2["k"][rows_p].T),
            "l2tv": np.ascontiguousarray(l2["v"][rows_n].T),
            "l2to": np.ascontiguousarray(l2["o"].T),
        }
        for nm in ("q", "k", "v", "o"):
            m[f"l1{nm}"] = l1[nm]
        in_maps.append(m)
    return in_maps


def _enable_ldw_opt():
    import concourse.bass_utils as _bu
    if getattr(_bu, "_ldw_patched", False):
        return
    _orig = _bu.run_command
    def _patched(argv, **kw):
        argv = ["--enable-ldw-opt=true" if a == "--enable-ldw-opt=false" else a
                for a in argv]
        return _orig(argv, **kw)
    _bu.run_command = _patched
    _bu._ldw_patched = True


def _install_trace_hook():
    """Provide antenv.axon_hooks (absent in this image) so trace=True works."""
    import types
    try:
        import antenv.axon_hooks  # noqa
        return
    except ImportError:
        pass
    try:
        from trn_agent_boot.trn_boot import _ntff_profile_via_ctypes
        hook = _ntff_profile_via_ctypes("/opt/axon/libaxon_pjrt.so")
        mod = types.ModuleType("antenv.axon_hooks")
        mod.get_axon_ntff_profile_hook = lambda: hook
        mod.set_axon_ntff_profile_hook = lambda h: None
        sys.modules["antenv.axon_hooks"] = mod
        import concourse.bass_utils as _bu
        _bu.upload_artifacts = lambda d: str(d)
    except Exception as e:
        print(f"trace hook install failed: {e}")


def kernel(**inputs):
    global LAST_EXEC_NS
    import os as _os
    if _os.environ.get("KLDW"):
        _enable_ldw_opt()
    if "nc" not in _CACHE:
        _CACHE["nc"] = _build()
    nc = _CACHE["nc"]
    in_maps = _shard_inputs(inputs)
    if TRACE:
        _install_trace_hook()
    res = run_bass_kernel_spmd(nc, in_maps, core_ids=list(range(NCORES)),
                               trace=TRACE)
    _CACHE["res"] = res
    LAST_EXEC_NS = res.exec_time_ns
    out = np.empty((B, S, D), dtype=np.float32)
    for c in range(NCORES):
        out[:, 512 * c:512 * (c + 1), :] = np.asarray(
            res.results[c]["out"], dtype=np.float32)
    return out
